# revision 27
# baseline (speedup 1.0000x reference)
"""KSparseFFTClassifier Trainium2 kernel.

Math: reference computes
    h   = x @ W_proj.T + b_proj                      (bs, 129)
    h  *= scale  (sqrt(2) on dims 1..64)
    out = IDFT65(h[:, :65]) + h[:, 65:] @ Ws.T       (bs, 16384)

The zero-padded orthonormal IDFT of the 65 nonzero frequency components is a
dense matmul against a (65, N) cos/sin basis; the DC row of that basis is the
constant 1/sqrt(N).  So with M = [cos/sin basis for h dims 1..64; Ws.T]
(128 x N):

    out[b, n] = hT[1:129, b] @ M[:, n] + (h[b, 0] + b0) / sqrt(N)

The kernel is HBM-DMA bound (the fp32 output write is 32 MiB/core), so the
read path runs in reduced precision and the 64 trig rows of M are generated
on device instead of loaded:
    m' = ((k_p*j + c2[p, chunk]) & (N-1)) ^ N/2,  row = Sin(2pi/N*m' - pi)
which equals cos/sin(2pi*k*n/N) exactly (the xor swaps halves so the -pi
recenters into the Sin table domain).  k_p*j is chunk-independent and computed
once; c2 is a tiny host-precomputed per-chunk phase table.  Only Ws.T (64 x N)
is read from HBM for mm2.  The sqrt(2)/sqrt(N) amplitude is folded into the
per-partition scale of the mm1 eviction.

Sharding: data-parallel over batch, 512 rows per core on 8 cores.
"""

import numpy as np

BS = 4096
IN_DIM = 2048
N = 16384
K = 32
SLACK = 64
NCORES = 8
BC = BS // NCORES        # 512 batch rows per core
P = 128
KT = IN_DIM // P         # 16 contraction tiles for matmul1
NCHUNK = 4096            # output column chunk (SBUF out tile free size)
NCH = N // NCHUNK        # 4
GC = 2048                # trig-generation chunk (free cols)
NGC = N // GC            # 8

MM1_DT = "float8e4"      # x dtype for matmul1 (weights stay bf16)
MM2_DT = "float8e4"      # hT / M dtype for matmul2

_NC_CACHE = {}


def _np_dt(name):
    import ml_dtypes
    return {
        "bfloat16": ml_dtypes.bfloat16,
        "float8e4": ml_dtypes.float8_e4m3,
        "float32": np.float32,
        "float32r": np.float32,
    }[name]


def _build_nc(mm1_name, mm2_name):
    import concourse.bacc as bacc
    import concourse.mybir as mybir
    import concourse.tile as tile

    f32 = mybir.dt.float32
    i32 = mybir.dt.int32
    bf16 = mybir.dt.bfloat16
    mm1 = getattr(mybir.dt, mm1_name)
    mm2 = getattr(mybir.dt, mm2_name)
    Alu = mybir.AluOpType
    Ident = mybir.ActivationFunctionType.Identity
    Sin = mybir.ActivationFunctionType.Sin

    nc = bacc.Bacc("TRN2", target_bir_lowering=False)

    xT = nc.dram_tensor("xT", [P, KT * BC], mm1, kind="ExternalInput")
    # w1t (128x2048) and w0 (128x16) packed in one tensor
    wc = nc.dram_tensor("wc", [P, KT * P + KT], bf16, kind="ExternalInput")
    wst = nc.dram_tensor("wst", [SLACK, N], mm2, kind="ExternalInput")
    # packed f32 consts: col0 = evict scale t, col1 = b*t, col2 = k_p,
    # col3 = -pi, col4..11 = per-chunk (k_p*n0+off_p) mod N, col12[0] = cst
    cc = nc.dram_tensor("cc", [P, 16], f32, kind="ExternalInput")
    out = nc.dram_tensor("out", [BC, N], f32, kind="ExternalOutput")

    TWO_PI_N = float(2.0 * np.pi / N)

    with tile.TileContext(nc) as tc:
        with (
            tc.tile_pool(name="wp", bufs=1) as wp,
            tc.tile_pool(name="xp", bufs=1) as xp,
            tc.tile_pool(name="mp", bufs=1) as mp,
            tc.tile_pool(name="gp", bufs=2) as gp,
            tc.tile_pool(name="hp", bufs=1) as hp,
            tc.tile_pool(name="op", bufs=3) as op,
            tc.tile_pool(name="ps", bufs=4, space="PSUM") as ps,
            tc.tile_pool(name="ps1", bufs=1, space="PSUM") as ps1,
            tc.tile_pool(name="ps2", bufs=1, space="PSUM") as ps2,
        ):
            # small consts + weights first (mm1 needs them before x lands)
            cc_sb = wp.tile([P, 16], f32, tag="cc")
            nc.sync.dma_start(out=cc_sb[:, :], in_=cc[:, :])
            wc_sb = wp.tile([P, KT * P + KT], bf16, tag="wc")
            nc.sync.dma_start(out=wc_sb[:, :], in_=wc[:, :])
            w1t_sb = wc_sb[:, 0:KT * P]
            w0_sb = wc_sb[:, KT * P:KT * P + KT]

            # x: two 1 MiB DMAs (16 KiB-contiguous rows per half) so mm1 can
            # start at the first half while descriptors stay large
            xg = xp.tile([P, KT * BC], mm1, tag="xg")
            HB = KT * BC // 2
            nc.sync.dma_start(out=xg[:, 0:HB], in_=xT[:, 0:HB])
            nc.sync.dma_start(out=xg[:, HB:2 * HB], in_=xT[:, HB:2 * HB])

            ones_sb = wp.tile([1, 1], f32, tag="ones")
            nc.vector.memset(ones_sb[:, :], 1.0)

            # M tiles: partitions 0..63 = generated trig rows, 64..127 = Ws.T
            mm = [
                mp.tile([P, NCHUNK], mm2, tag=f"m{ti}", name=f"m{ti}")
                for ti in range(NCH)
            ]

            # Ws.T staged via ONE full-row DMA (16 KiB descriptors; a per-tile
            # slice would emit slow 4 KiB ones), then SBUF->SBUF into m tiles
            wst_sb = wp.tile([SLACK, N], mm2, tag="wst_sb")
            nc.sync.dma_start(out=wst_sb[:, :], in_=wst[:, :])

            it_sb = wp.tile([SLACK, GC], i32, tag="it")
            nc.gpsimd.iota(it_sb[:, :], pattern=[[1, GC]], base=0,
                           channel_multiplier=0)
            kn_sb = wp.tile([SLACK, GC], i32, tag="kn")
            nc.vector.tensor_scalar(
                kn_sb[:, :], it_sb[:, :], cc_sb[0:SLACK, 2:3], None, Alu.mult,
            )

            def load_m(ti):
                nc.sync.dma_start(
                    out=mm[ti][64:128, :],
                    in_=wst_sb[:, ti * NCHUNK:(ti + 1) * NCHUNK],
                )

            gen_ph = {}

            def gen_m_add(ti, c):
                gc = ti * (NCHUNK // GC) + c
                ph = gp.tile([SLACK, GC], i32, tag="ph")
                nc.vector.tensor_scalar(
                    ph[:, :], kn_sb[:, :], cc_sb[0:SLACK, 4 + gc:5 + gc],
                    None, Alu.add,
                )
                gen_ph[(ti, c)] = ph

            def gen_m_trig(ti, c):
                ph = gen_ph.pop((ti, c))
                md = gp.tile([SLACK, GC], i32, tag="md")
                nc.vector.tensor_scalar(
                    md[:, :], ph[:, :], int(N - 1), int(N // 2),
                    Alu.bitwise_and, Alu.bitwise_xor,
                )
                nc.scalar.activation(
                    mm[ti][0:64, c * GC:(c + 1) * GC], md[:, :], Sin,
                    bias=cc_sb[0:SLACK, 3:4], scale=TWO_PI_N,
                )

            load_m(0)
            for c in range(NCHUNK // GC):
                gen_m_add(0, c)
                gen_m_trig(0, c)

            # matmul1: hT[d, b] for d = h dims 1..128
            hT_ps = ps1.tile([P, BC], f32, tag="hT")
            for kt in range(KT):
                nc.tensor.matmul(
                    hT_ps[:, :],
                    lhsT=w1t_sb[:, kt * P:(kt + 1) * P],
                    rhs=xg[:, kt * BC:(kt + 1) * BC],
                    start=(kt == 0),
                    stop=(kt == KT - 1),
                )
            # evict with per-partition fold: hT = (h + b) * t_p
            # (t_p = sqrt2/sqrtN on trig rows, 1 on slack rows)
            hT_sb = hp.tile([P, BC], mm2, tag="hT_sb")
            nc.vector.tensor_scalar(
                hT_sb[:, :], hT_ps[:, :], cc_sb[:, 0:1], cc_sb[:, 1:2],
                Alu.mult, Alu.add,
            )

            # dc row: h dim 0 (as (1, BC)), then PE-transpose to (P, 4)
            dcr_ps = ps2.tile([1, BC], f32, tag="dcr")
            for kt in range(KT):
                nc.tensor.matmul(
                    dcr_ps[:, :],
                    lhsT=w0_sb[:, kt:kt + 1],
                    rhs=xg[:, kt * BC:(kt + 1) * BC],
                    start=(kt == 0),
                    stop=(kt == KT - 1),
                )
            dcr_sb = hp.tile([1, BC], f32, tag="dcr_sb")
            nc.scalar.activation(
                dcr_sb[:, :], dcr_ps[:, :], Ident,
                bias=cc_sb[0:1, 12:13], scale=float(1.0 / np.sqrt(N)),
            )
            dc_sb = hp.tile([P, BC // P], f32, tag="dc_sb")
            for j in range(BC // P):
                dcc_ps = ps2.tile([P, 1], f32, tag="dcc")
                nc.tensor.matmul(
                    dcc_ps[:, :],
                    lhsT=dcr_sb[0:1, j * P:(j + 1) * P],
                    rhs=ones_sb[0:1, 0:1],
                    start=True,
                    stop=True,
                )
                nc.scalar.copy(dc_sb[:, j:j + 1], dcc_ps[:, :])

            # matmul2 + DC bias-add eviction + store (2048-col stores so the
            # first write launches early).  The next M tile's Ws.T copy and
            # trig generation are spread across this tile's j-blocks so each
            # engine's in-order stream never starves the output pipeline.
            ev = 0
            for ti in range(NCH):
                for j in range(BC // P):
                    if ti + 1 < NCH:
                        # next tile fully generated by end of j==1: engines
                        # have catch-up slack in j==2/3 and the ti handoff
                        # never stalls
                        if j == 0:
                            load_m(ti + 1)
                            gen_m_add(ti + 1, 0)
                            gen_m_trig(ti + 1, 0)
                        elif j == 1:
                            gen_m_add(ti + 1, 1)
                            gen_m_trig(ti + 1, 1)
                    # 2048-col stores while ramping (first tile), then 4096
                    nhalf = 2 if ti == 0 else 1
                    for half in range(nhalf):
                        fs = NCHUNK // nhalf
                        ob = op.tile([P, fs], f32, tag=f"ob{nhalf}", name="ob")
                        for s4 in range(fs // 512):
                            s = half * (fs // 512) + s4
                            pt = ps.tile([P, 512], f32, tag="mm2")
                            nc.tensor.matmul(
                                pt[:, :],
                                lhsT=hT_sb[:, j * P:(j + 1) * P],
                                rhs=mm[ti][:, s * 512:(s + 1) * 512],
                                start=True,
                                stop=True,
                            )
                            dst = ob[:, s4 * 512:(s4 + 1) * 512]
                            # ~7:9 DVE:ACT split (both also carry gen work)
                            if (ev % 16) in (1, 3, 5, 7, 9, 11, 13):
                                nc.vector.tensor_scalar_add(dst, pt[:, :], dc_sb[:, j:j + 1])
                            else:
                                nc.scalar.add(dst, pt[:, :], dc_sb[:, j:j + 1])
                            ev += 1
                        nc.sync.dma_start(
                            out=out[j * P:(j + 1) * P,
                                    ti * NCHUNK + half * fs:
                                    ti * NCHUNK + (half + 1) * fs],
                            in_=ob[:, :],
                        )
    nc.compile()
    return nc


def _get_nc():
    key = (MM1_DT, MM2_DT)
    if key not in _NC_CACHE:
        _NC_CACHE[key] = _build_nc(*key)
    return _NC_CACHE[key]


def _host_pack(x, W_proj, b_proj, Ws):
    import ml_dtypes
    dt1 = _np_dt(MM1_DT)
    dt2 = _np_dt(MM2_DT)
    dtw = ml_dtypes.bfloat16
    SQRT2 = np.float64(np.sqrt(np.float32(2.0)))
    isqn = 1.0 / np.sqrt(np.float64(N))

    wst = np.ascontiguousarray(Ws.T.astype(dt2))                  # (64, N)

    w1 = W_proj[1:P + 1]                                          # (128, 2048)
    w1t = w1.T.reshape(KT, P, P).transpose(1, 0, 2).reshape(P, KT * P)
    w0 = W_proj[0].reshape(KT, P).T                               # (128, 16)
    wc = np.ascontiguousarray(
        np.concatenate([w1t, w0], axis=1).astype(dtw)             # (128, 2064)
    )

    # packed f32 consts (128, 16)
    t = np.ones((P,), np.float64)
    t[:2 * K] = SQRT2 * isqn
    b = b_proj[1:P + 1].astype(np.float64)
    kp = (np.arange(SLACK) // 2 + 1).astype(np.int64)
    off = np.where(np.arange(SLACK) % 2 == 0, N // 4, 0).astype(np.int64)
    n0 = (np.arange(NGC) * GC).astype(np.int64)
    c2 = (kp[:, None] * n0[None, :] + off[:, None]) % N           # (64, 8)
    cc = np.zeros((P, 16), np.float32)
    cc[:, 0] = t
    cc[:, 1] = b * t
    cc[:SLACK, 2] = kp
    cc[:SLACK, 3] = -np.pi
    cc[:SLACK, 4:4 + NGC] = c2
    cc[0, 12] = b_proj[0] * isqn
    cc = np.ascontiguousarray(cc)

    xts = []
    for c in range(NCORES):
        xc = x[c * BC:(c + 1) * BC]                               # (512, 2048)
        xt = np.ascontiguousarray(
            xc.T.reshape(KT, P, BC).transpose(1, 0, 2).reshape(P, KT * BC).astype(dt1)
        )
        xts.append(xt)
    return wst, wc, cc, xts


def kernel(x, W_proj, b_proj, Ws, _trace=False, _tmpdir=None):
    from concourse import bass_utils

    x = np.ascontiguousarray(x, np.float32)
    W_proj = np.ascontiguousarray(W_proj, np.float32)
    b_proj = np.ascontiguousarray(b_proj, np.float32)
    Ws = np.ascontiguousarray(Ws, np.float32)

    wst, wc, cc, xts = _host_pack(x, W_proj, b_proj, Ws)
    nc = _get_nc()

    in_maps = [
        {"xT": xts[c], "wc": wc, "wst": wst, "cc": cc}
        for c in range(NCORES)
    ]
    kw = {}
    if _trace:
        kw = dict(trace=True, tmpdir=_tmpdir, trace_cores=[0])
    res = bass_utils.run_bass_kernel_spmd(nc, in_maps, core_ids=list(range(NCORES)), **kw)
    out = np.concatenate([r["out"] for r in res.results], axis=0)
    if _trace:
        return out, res
    return out


# revision 30
# speedup vs baseline: 1.1359x; 1.1359x over previous
"""KSparseFFTClassifier Trainium2 kernel.

Math: reference computes
    h   = x @ W_proj.T + b_proj                      (bs, 129)
    h  *= scale  (sqrt(2) on dims 1..64)
    out = IDFT65(h[:, :65]) + h[:, 65:] @ Ws.T       (bs, 16384)

The zero-padded orthonormal IDFT of the 65 nonzero frequency components is a
dense matmul against a (65, N) cos/sin basis; the DC row of that basis is the
constant 1/sqrt(N).  So with M = [cos/sin basis for h dims 1..64; Ws.T]
(128 x N):

    out[b, n] = hT[1:129, b] @ M[:, n] + (h[b, 0] + b0) / sqrt(N)

The kernel is HBM-DMA bound (the fp32 output write is 32 MiB/core), so the
read path runs in reduced precision and the 64 trig rows of M are generated
on device instead of loaded:
    m' = ((k_p*j + c2[p, chunk]) & (N-1)) ^ N/2,  row = Sin(2pi/N*m' - pi)
which equals cos/sin(2pi*k*n/N) exactly (the xor swaps halves so the -pi
recenters into the Sin table domain).  k_p*j is chunk-independent and computed
once; c2 is a tiny host-precomputed per-chunk phase table.  Only Ws.T (64 x N)
is read from HBM for mm2.  The sqrt(2)/sqrt(N) amplitude is folded into the
per-partition scale of the mm1 eviction.

Sharding: data-parallel over batch, 512 rows per core on 8 cores.
"""

import numpy as np

BS = 4096
IN_DIM = 2048
N = 16384
K = 32
SLACK = 64
NCORES = 8
BC = BS // NCORES        # 512 batch rows per core
P = 128
KT = IN_DIM // P         # 16 contraction tiles for matmul1
NCHUNK = 4096            # output column chunk (SBUF out tile free size)
NCH = N // NCHUNK        # 4
GC = 2048                # trig-generation chunk (free cols)
NGC = N // GC            # 8

MM1_DT = "float8e4"      # x dtype for matmul1 (weights stay bf16)
MM2_DT = "float8e4"      # hT / M dtype for matmul2

_NC_CACHE = {}


def _np_dt(name):
    import ml_dtypes
    return {
        "bfloat16": ml_dtypes.bfloat16,
        "float8e4": ml_dtypes.float8_e4m3,
        "float32": np.float32,
        "float32r": np.float32,
    }[name]


def _build_nc(mm1_name, mm2_name):
    import concourse.bacc as bacc
    import concourse.mybir as mybir
    import concourse.tile as tile

    f32 = mybir.dt.float32
    i32 = mybir.dt.int32
    bf16 = mybir.dt.bfloat16
    mm1 = getattr(mybir.dt, mm1_name)
    mm2 = getattr(mybir.dt, mm2_name)
    Alu = mybir.AluOpType
    Ident = mybir.ActivationFunctionType.Identity
    Sin = mybir.ActivationFunctionType.Sin

    nc = bacc.Bacc("TRN2", target_bir_lowering=False)

    xT = nc.dram_tensor("xT", [P, KT * BC], mm1, kind="ExternalInput")
    # w1t (128x2048) and w0 (128x16) packed in one tensor
    wc = nc.dram_tensor("wc", [P, KT * P + KT], bf16, kind="ExternalInput")
    wst = nc.dram_tensor("wst", [SLACK, N], mm2, kind="ExternalInput")
    # packed f32 consts: col0 = evict scale t, col1 = b*t, col2 = k_p,
    # col3 = -pi, col4..11 = per-chunk (k_p*n0+off_p) mod N, col12[0] = cst
    cc = nc.dram_tensor("cc", [P, 16], f32, kind="ExternalInput")
    out = nc.dram_tensor("out", [BC, N], f32, kind="ExternalOutput")

    TWO_PI_N = float(2.0 * np.pi / N)

    with tile.TileContext(nc) as tc:
        with (
            tc.tile_pool(name="wp", bufs=1) as wp,
            tc.tile_pool(name="xp", bufs=1) as xp,
            tc.tile_pool(name="mp", bufs=1) as mp,
            tc.tile_pool(name="gp", bufs=2) as gp,
            tc.tile_pool(name="hp", bufs=1) as hp,
            tc.tile_pool(name="op", bufs=3) as op,
            tc.tile_pool(name="ps", bufs=4, space="PSUM") as ps,
            tc.tile_pool(name="ps1", bufs=1, space="PSUM") as ps1,
            tc.tile_pool(name="ps2", bufs=1, space="PSUM") as ps2,
        ):
            # small consts + weights first (mm1 needs them before x lands)
            cc_sb = wp.tile([P, 16], f32, tag="cc")
            nc.sync.dma_start(out=cc_sb[:, :], in_=cc[:, :])
            wc_sb = wp.tile([P, KT * P + KT], bf16, tag="wc")
            nc.sync.dma_start(out=wc_sb[:, :], in_=wc[:, :])
            w1t_sb = wc_sb[:, 0:KT * P]
            w0_sb = wc_sb[:, KT * P:KT * P + KT]

            # x: two 1 MiB DMAs (16 KiB-contiguous rows per half) so mm1 can
            # start at the first half while descriptors stay large
            # x on the Scalar-engine HWDGE ring: parallel DGE path, so the
            # load starts while Sync still issues the consts
            xg = xp.tile([P, KT * BC], mm1, tag="xg")
            HB = KT * BC // 2
            nc.scalar.dma_start(out=xg[:, 0:HB], in_=xT[:, 0:HB])
            nc.scalar.dma_start(out=xg[:, HB:2 * HB], in_=xT[:, HB:2 * HB])

            ones_sb = wp.tile([1, 1], f32, tag="ones")
            nc.vector.memset(ones_sb[:, :], 1.0)

            # M tiles: partitions 0..63 = generated trig rows, 64..127 = Ws.T
            mm = [
                mp.tile([P, NCHUNK], mm2, tag=f"m{ti}", name=f"m{ti}")
                for ti in range(NCH)
            ]

            # Ws.T staged via ONE full-row DMA (16 KiB descriptors; a per-tile
            # slice would emit slow 4 KiB ones), then SBUF->SBUF into m tiles
            wst_sb = wp.tile([SLACK, N], mm2, tag="wst_sb")
            nc.scalar.dma_start(out=wst_sb[:, :], in_=wst[:, :])

            it_sb = wp.tile([SLACK, GC], i32, tag="it")
            nc.gpsimd.iota(it_sb[:, :], pattern=[[1, GC]], base=0,
                           channel_multiplier=0)
            kn_sb = wp.tile([SLACK, GC], i32, tag="kn")
            nc.vector.tensor_scalar(
                kn_sb[:, :], it_sb[:, :], cc_sb[0:SLACK, 2:3], None, Alu.mult,
            )

            def load_m(ti):
                # SBUF->SBUF on the gpsimd SWDGE ring: keeps the Sync queue a
                # pure output-write stream and doesn't touch HBM bandwidth
                nc.gpsimd.dma_start(
                    out=mm[ti][64:128, :],
                    in_=wst_sb[:, ti * NCHUNK:(ti + 1) * NCHUNK],
                )

            gen_ph = {}

            def gen_m_add(ti, c):
                gc = ti * (NCHUNK // GC) + c
                ph = gp.tile([SLACK, GC], i32, tag="ph")
                nc.vector.tensor_scalar(
                    ph[:, :], kn_sb[:, :], cc_sb[0:SLACK, 4 + gc:5 + gc],
                    None, Alu.add,
                )
                gen_ph[(ti, c)] = ph

            def gen_m_trig(ti, c):
                ph = gen_ph.pop((ti, c))
                md = gp.tile([SLACK, GC], i32, tag="md")
                nc.vector.tensor_scalar(
                    md[:, :], ph[:, :], int(N - 1), int(N // 2),
                    Alu.bitwise_and, Alu.bitwise_xor,
                )
                nc.scalar.activation(
                    mm[ti][0:64, c * GC:(c + 1) * GC], md[:, :], Sin,
                    bias=cc_sb[0:SLACK, 3:4], scale=TWO_PI_N,
                )

            load_m(0)
            for c in range(NCHUNK // GC):
                gen_m_add(0, c)
                gen_m_trig(0, c)

            # matmul1: hT[d, b] for d = h dims 1..128
            hT_ps = ps1.tile([P, BC], f32, tag="hT")
            for kt in range(KT):
                nc.tensor.matmul(
                    hT_ps[:, :],
                    lhsT=w1t_sb[:, kt * P:(kt + 1) * P],
                    rhs=xg[:, kt * BC:(kt + 1) * BC],
                    start=(kt == 0),
                    stop=(kt == KT - 1),
                )
            # evict with per-partition fold: hT = (h + b) * t_p
            # (t_p = sqrt2/sqrtN on trig rows, 1 on slack rows)
            hT_sb = hp.tile([P, BC], mm2, tag="hT_sb")
            nc.vector.tensor_scalar(
                hT_sb[:, :], hT_ps[:, :], cc_sb[:, 0:1], cc_sb[:, 1:2],
                Alu.mult, Alu.add,
            )

            # dc row: h dim 0 (as (1, BC)), then PE-transpose to (P, 4)
            dcr_ps = ps2.tile([1, BC], f32, tag="dcr")
            for kt in range(KT):
                nc.tensor.matmul(
                    dcr_ps[:, :],
                    lhsT=w0_sb[:, kt:kt + 1],
                    rhs=xg[:, kt * BC:(kt + 1) * BC],
                    start=(kt == 0),
                    stop=(kt == KT - 1),
                )
            dcr_sb = hp.tile([1, BC], f32, tag="dcr_sb")
            nc.scalar.activation(
                dcr_sb[:, :], dcr_ps[:, :], Ident,
                bias=cc_sb[0:1, 12:13], scale=float(1.0 / np.sqrt(N)),
            )
            dc_sb = hp.tile([P, BC // P], f32, tag="dc_sb")
            for j in range(BC // P):
                dcc_ps = ps2.tile([P, 1], f32, tag="dcc")
                nc.tensor.matmul(
                    dcc_ps[:, :],
                    lhsT=dcr_sb[0:1, j * P:(j + 1) * P],
                    rhs=ones_sb[0:1, 0:1],
                    start=True,
                    stop=True,
                )
                nc.scalar.copy(dc_sb[:, j:j + 1], dcc_ps[:, :])

            # matmul2 + DC bias-add eviction + store (2048-col stores so the
            # first write launches early).  The next M tile's Ws.T copy and
            # trig generation are spread across this tile's j-blocks so each
            # engine's in-order stream never starves the output pipeline.
            ev = 0
            for ti in range(NCH):
                for j in range(BC // P):
                    if ti + 1 < NCH:
                        # next tile fully generated by end of j==1: engines
                        # have catch-up slack in j==2/3 and the ti handoff
                        # never stalls
                        if j == 0:
                            load_m(ti + 1)
                            gen_m_add(ti + 1, 0)
                            gen_m_trig(ti + 1, 0)
                        elif j == 1:
                            gen_m_add(ti + 1, 1)
                            gen_m_trig(ti + 1, 1)
                    # 2048-col stores while ramping (first tile), then 4096
                    nhalf = 2 if ti == 0 else 1
                    for half in range(nhalf):
                        fs = NCHUNK // nhalf
                        ob = op.tile([P, fs], f32, tag=f"ob{nhalf}", name="ob")
                        for s4 in range(fs // 512):
                            s = half * (fs // 512) + s4
                            pt = ps.tile([P, 512], f32, tag="mm2")
                            nc.tensor.matmul(
                                pt[:, :],
                                lhsT=hT_sb[:, j * P:(j + 1) * P],
                                rhs=mm[ti][:, s * 512:(s + 1) * 512],
                                start=True,
                                stop=True,
                            )
                            dst = ob[:, s4 * 512:(s4 + 1) * 512]
                            # ~7:9 DVE:ACT split (both also carry gen work)
                            if (ev % 16) in (1, 3, 5, 7, 9, 11, 13):
                                nc.vector.tensor_scalar_add(dst, pt[:, :], dc_sb[:, j:j + 1])
                            else:
                                nc.scalar.add(dst, pt[:, :], dc_sb[:, j:j + 1])
                            ev += 1
                        nc.sync.dma_start(
                            out=out[j * P:(j + 1) * P,
                                    ti * NCHUNK + half * fs:
                                    ti * NCHUNK + (half + 1) * fs],
                            in_=ob[:, :],
                        )
    nc.compile()
    return nc


def _get_nc():
    key = (MM1_DT, MM2_DT)
    if key not in _NC_CACHE:
        _NC_CACHE[key] = _build_nc(*key)
    return _NC_CACHE[key]


def _host_pack(x, W_proj, b_proj, Ws):
    import ml_dtypes
    dt1 = _np_dt(MM1_DT)
    dt2 = _np_dt(MM2_DT)
    dtw = ml_dtypes.bfloat16
    SQRT2 = np.float64(np.sqrt(np.float32(2.0)))
    isqn = 1.0 / np.sqrt(np.float64(N))

    wst = np.ascontiguousarray(Ws.T.astype(dt2))                  # (64, N)

    w1 = W_proj[1:P + 1]                                          # (128, 2048)
    w1t = w1.T.reshape(KT, P, P).transpose(1, 0, 2).reshape(P, KT * P)
    w0 = W_proj[0].reshape(KT, P).T                               # (128, 16)
    wc = np.ascontiguousarray(
        np.concatenate([w1t, w0], axis=1).astype(dtw)             # (128, 2064)
    )

    # packed f32 consts (128, 16)
    t = np.ones((P,), np.float64)
    t[:2 * K] = SQRT2 * isqn
    b = b_proj[1:P + 1].astype(np.float64)
    kp = (np.arange(SLACK) // 2 + 1).astype(np.int64)
    off = np.where(np.arange(SLACK) % 2 == 0, N // 4, 0).astype(np.int64)
    n0 = (np.arange(NGC) * GC).astype(np.int64)
    c2 = (kp[:, None] * n0[None, :] + off[:, None]) % N           # (64, 8)
    cc = np.zeros((P, 16), np.float32)
    cc[:, 0] = t
    cc[:, 1] = b * t
    cc[:SLACK, 2] = kp
    cc[:SLACK, 3] = -np.pi
    cc[:SLACK, 4:4 + NGC] = c2
    cc[0, 12] = b_proj[0] * isqn
    cc = np.ascontiguousarray(cc)

    xts = []
    for c in range(NCORES):
        xc = x[c * BC:(c + 1) * BC]                               # (512, 2048)
        xt = np.ascontiguousarray(
            xc.T.reshape(KT, P, BC).transpose(1, 0, 2).reshape(P, KT * BC).astype(dt1)
        )
        xts.append(xt)
    return wst, wc, cc, xts


def kernel(x, W_proj, b_proj, Ws, _trace=False, _tmpdir=None):
    from concourse import bass_utils

    x = np.ascontiguousarray(x, np.float32)
    W_proj = np.ascontiguousarray(W_proj, np.float32)
    b_proj = np.ascontiguousarray(b_proj, np.float32)
    Ws = np.ascontiguousarray(Ws, np.float32)

    wst, wc, cc, xts = _host_pack(x, W_proj, b_proj, Ws)
    nc = _get_nc()

    in_maps = [
        {"xT": xts[c], "wc": wc, "wst": wst, "cc": cc}
        for c in range(NCORES)
    ]
    kw = {}
    if _trace:
        kw = dict(trace=True, tmpdir=_tmpdir, trace_cores=[0])
    res = bass_utils.run_bass_kernel_spmd(nc, in_maps, core_ids=list(range(NCORES)), **kw)
    out = np.concatenate([r["out"] for r in res.results], axis=0)
    if _trace:
        return out, res
    return out
